# revision 16
# baseline (speedup 1.0000x reference)
"""BiDAF attention Trainium2 kernel.

Full-input contract: kernel(**inputs) takes the unsharded tensors
(context [16,2048,128], query [16,128,128], W [384],
context_mask [16,2048] i32, query_mask [16,128] i32) and returns
G = [16, 2048, 512] f32.

Sharding: data-parallel over batch across 8 NeuronCores (2 batches/core).
Each batch's math is fully local to a core, so no collectives.
"""

import sys

sys.path.insert(0, "/opt/trn_rl_repo")

import numpy as np

import concourse.bass as bass
import concourse.tile as tile
from concourse import mybir
from concourse.masks import make_identity
from concourse.vector_clock import ScopedClock

B, C_LEN, Q_LEN, H = 16, 2048, 128, 128
NEG = -1000000000.0
N_CORES = 8
B_PER_CORE = B // N_CORES          # 2
N_CTILES = C_LEN // 128            # 16
F32 = mybir.dt.float32
I32 = mybir.dt.int32

MAX_WAITS_PER_INST = 1


def _split_excess_waits(nc, insts):
    """Hoist all but one sync wait per instruction onto preceding nops.

    The walrus build in this container rejects >1 sync wait on an
    instruction's descriptor, while Tile's sem assignment freely attaches
    several. A nop on the same engine right before the instruction stalls
    the engine identically.
    """
    out = []
    for inst in insts:
        si = getattr(inst, "sync_info", None)
        waits = list(si.on_wait) if si is not None and si.on_wait else []
        if len(waits) > MAX_WAITS_PER_INST and type(inst).__name__.startswith("Inst"):
            extra = waits[: -MAX_WAITS_PER_INST or None]
            keep = waits[-MAX_WAITS_PER_INST:]
            for i in range(0, len(extra), MAX_WAITS_PER_INST):
                out.append(
                    mybir.InstNoOp(
                        name=nc.get_next_instruction_name(),
                        sync_info=mybir.SyncInfo(
                            on_wait=extra[i : i + MAX_WAITS_PER_INST], on_update=[]
                        ),
                        bass_nofuse=True,
                        engine=inst.engine,
                    )
                )
            inst.sync_info = mybir.SyncInfo(
                on_wait=keep, on_update=list(si.on_update or [])
            )
        out.append(inst)
    return out


class SplitDrainTileContext(tile.TileContext):
    """TileContext whose tail drain splits its sem waits across SP nops.

    The walrus build in this container rejects more than one sync wait on a
    TPB_CTRL instruction; the stock tail drain carries one wait per live proc.
    """

    def _lower_ordered_insts(self, ordered):
        for bb_name in list(ordered.keys()):
            ordered[bb_name] = _split_excess_waits(self.nc, ordered[bb_name])
        return super()._lower_ordered_insts(ordered)

    def _drain_and_barrier(self, tick_clock, wait_clock):
        nc = self.nc
        drain_inst = nc.sync.drain()
        wait_clock.add_sem_waits(
            drain_inst.ins, ScopedClock({None: tick_clock.global_clock})
        )
        si = drain_inst.ins.sync_info
        waits = list(si.on_wait) if si is not None and si.on_wait else []
        if waits:
            drain_inst.ins.sync_info = mybir.SyncInfo(
                on_wait=[], on_update=list(si.on_update or [])
            )
            for i in range(0, len(waits), MAX_WAITS_PER_INST):
                nop = nc.sync.nop()
                nop.ins.sync_info = mybir.SyncInfo(
                    on_wait=waits[i : i + MAX_WAITS_PER_INST], on_update=[]
                )
        nc.all_engine_barrier()
        assert self.sems is not None
        popped = nc._tile_sem_poison_stack.pop()
        assert popped is self._sem_poison
        nc.clear_and_free_semaphores(list(self.sems.allocated().values()))
        nc.all_engine_barrier()


def build_nc() -> bass.Bass:
    nc = bass.Bass()
    ctx_d = nc.dram_tensor("context", [B_PER_CORE, C_LEN, H], F32, kind="ExternalInput")
    qry_d = nc.dram_tensor("query", [B_PER_CORE, Q_LEN, H], F32, kind="ExternalInput")
    w_d = nc.dram_tensor("W", [3 * H], F32, kind="ExternalInput")
    cm_d = nc.dram_tensor("context_mask", [B_PER_CORE, C_LEN], I32, kind="ExternalInput")
    qm_d = nc.dram_tensor("query_mask", [B_PER_CORE, Q_LEN], I32, kind="ExternalInput")
    g_d = nc.dram_tensor("G", [B_PER_CORE, C_LEN, 4 * H], F32, kind="ExternalOutput")

    F32R = mybir.dt.float32r
    NGRP = 4                      # tile groups per batch
    GS = N_CTILES // NGRP         # tiles per group (4)

    from contextlib import ExitStack

    with SplitDrainTileContext(nc) as tc, ExitStack() as es:
        consts = es.enter_context(tc.tile_pool(name="consts", bufs=1))
        batchp = es.enter_context(tc.tile_pool(name="batchp", bufs=2))
        work = es.enter_context(tc.tile_pool(name="work", bufs=4))
        gpool = es.enter_context(tc.tile_pool(name="gpool", bufs=2))
        pmm = es.enter_context(tc.tile_pool(name="pmm", bufs=2, space="PSUM"))
        ptrans = es.enter_context(tc.tile_pool(name="ptrans", bufs=2, space="PSUM"))
        pmisc = es.enter_context(tc.tile_pool(name="pmisc", bufs=2, space="PSUM"))
        ptt = es.enter_context(tc.tile_pool(name="ptt", bufs=2, space="PSUM"))

        identity = consts.tile([128, 128], F32)
        make_identity(nc, identity)
        ones_row = consts.tile([1, 128], F32)
        nc.vector.memset(ones_row, 1.0)
        ones_col = consts.tile([128, 1], F32)
        nc.vector.memset(ones_col, 1.0)
        neg_tile = consts.tile([128, N_CTILES], F32)
        nc.vector.memset(neg_tile, NEG)
        w_c = consts.tile([128, 1], F32)
        nc.gpsimd.dma_start(out=w_c, in_=w_d[0:H].rearrange("(h o) -> h o", o=1))
        w_q = consts.tile([128, 1], F32)
        nc.gpsimd.dma_start(out=w_q, in_=w_d[H : 2 * H].rearrange("(h o) -> h o", o=1))
        w_cq = consts.tile([128, 1], F32)
        nc.gpsimd.dma_start(out=w_cq, in_=w_d[2 * H : 3 * H].rearrange("(h o) -> h o", o=1))

        for b in range(B_PER_CORE):
            # ---- per-batch prelims ----
            # qrhs129 = [query | 1]; the moving operand of the c2q matmul
            # (the ones column turns into the softmax denominator)
            qrhs129 = batchp.tile([128, 129], F32, tag="qrhs129")
            query = qrhs129[:, 0:128]
            nc.gpsimd.dma_start(out=query, in_=qry_d[b])
            nc.vector.memset(qrhs129[:, 128:129], 1.0)

            ps_qt = ptrans.tile([128, 128], F32, tag="tr")
            nc.tensor.transpose(ps_qt, query, identity)
            queryT = batchp.tile([128, 128], F32, tag="queryT")
            nc.scalar.copy(out=queryT, in_=ps_qt)

            # rhs129 = [queryT * w_cq | w_c]  (moving operand of the S matmul;
            # its first 128 cols are also the T^T stationary operand)
            rhs129 = batchp.tile([128, 129], F32, tag="rhs129")
            qTw = rhs129[:, 0:128]
            nc.vector.tensor_scalar_mul(qTw, queryT, w_cq)
            nc.vector.tensor_copy(out=rhs129[:, 128:129], in_=w_c)

            # sqm as a column [q, 1]:  s_q + (qm - 1) * 1e9
            ps_sqc = pmisc.tile([128, 1], F32, tag="misc")
            nc.tensor.matmul(ps_sqc, queryT, w_q, start=True, stop=True)
            qm_col = batchp.tile([128, 1], I32, tag="qm_col")
            nc.gpsimd.dma_start(out=qm_col, in_=qm_d[b].rearrange("(q o) -> q o", o=1))
            qoff = batchp.tile([128, 1], F32, tag="qoff")
            nc.vector.tensor_copy(out=qoff, in_=qm_col)
            nc.vector.tensor_scalar(
                out=qoff,
                in0=qoff,
                scalar1=1.0,
                scalar2=-NEG,
                op0=mybir.AluOpType.subtract,
                op1=mybir.AluOpType.mult,
            )
            sqm_col = batchp.tile([128, 1], F32, tag="sqm_col")
            nc.vector.tensor_add(out=sqm_col, in0=ps_sqc, in1=qoff)

            # sqm as a row, padded to 129 cols; accumulated into the S matmul
            # via a k=1 PSUM-accumulate so T already carries s_q + mask
            ps_sqr = ptrans.tile([1, 128], F32, tag="tr")
            nc.tensor.transpose(ps_sqr, sqm_col, identity)
            srow129 = batchp.tile([1, 129], F32, tag="srow129")
            nc.vector.memset(srow129[:, 128:129], 0.0)
            nc.vector.tensor_copy(out=srow129[:, 0:128], in_=ps_sqr)

            cm_tile = batchp.tile([128, N_CTILES], I32, tag="cm_tile")
            nc.gpsimd.dma_start(
                out=cm_tile, in_=cm_d[b].rearrange("(t p) -> p t", p=128)
            )

            m_buf = batchp.tile([128, N_CTILES], F32, tag="m_buf")
            sc_buf = batchp.tile([128, N_CTILES], F32, tag="sc_buf")
            ctxT_buf = batchp.tile([128, C_LEN], F32, tag="ctxT_buf")
            eT_buf = batchp.tile([128, C_LEN], F32, tag="eT_buf")

            # one batch-wide G buffer [p, t, f]; context lands in f=0:128 in
            # two half-batch DMAs (c = t*128 + p) so compute starts earlier
            gbuf = gpool.tile([128, N_CTILES, 4 * H], F32, tag="g")
            ctx_view = ctx_d[b].rearrange("(t p) h -> p t h", p=128)
            hh = N_CTILES // 2
            nc.gpsimd.dma_start(out=gbuf[:, 0:hh, 0:128], in_=ctx_view[:, 0:hh, :])
            nc.gpsimd.dma_start(out=gbuf[:, hh:, 0:128], in_=ctx_view[:, hh:, :])
            g_view = g_d[b].rearrange("(t p) f -> p t f", p=128)

            for g in range(NGRP):
                # ---- phase A per tile: transpose ctx, S matmul, row-max ----
                for t in range(g * GS, (g + 1) * GS):
                    ctx_v = gbuf[:, t, 0:128]
                    ps_ct = ptrans.tile([128, 128], F32, tag="tr")
                    nc.tensor.transpose(ps_ct, ctx_v, identity)
                    ctxT = ctxT_buf[:, t * 128 : (t + 1) * 128]
                    nc.any.tensor_copy(out=ctxT, in_=ps_ct)

                    ps_T = pmm.tile([128, 129], F32, tag="mm")
                    nc.tensor.matmul(ps_T, ctxT, rhs129, start=True, stop=False)
                    nc.tensor.matmul(ps_T, ones_row, srow129, start=False, stop=True)
                    nc.vector.tensor_copy(
                        out=sc_buf[:, t : t + 1], in_=ps_T[:, 128:129]
                    )
                    nc.vector.reduce_max(
                        out=m_buf[:, t : t + 1],
                        in_=ps_T[:, 0:128],
                        axis=mybir.AxisListType.X,
                    )

                # ---- T^T chunk (fp32r) + exp -> eT ----
                c0 = g * GS * 128
                ps_tt = ptt.tile([128, GS * 128], F32, tag="tt")
                nc.tensor.matmul(
                    ps_tt,
                    qTw.bitcast(F32R),
                    ctxT_buf[:, c0 : c0 + GS * 128].bitcast(F32R),
                    start=True,
                    stop=True,
                )
                nc.scalar.activation(
                    out=eT_buf[:, c0 : c0 + GS * 128],
                    in_=ps_tt,
                    func=mybir.ActivationFunctionType.Exp,
                    bias=sqm_col,
                )

                # ---- phase B per tile: c2q, normalize, ctx*c2q ----
                for t in range(g * GS, (g + 1) * GS):
                    ctx_v = gbuf[:, t, 0:128]
                    ps_c2q = pmm.tile([128, 129], F32, tag="mm")
                    nc.tensor.matmul(
                        ps_c2q,
                        eT_buf[:, t * 128 : (t + 1) * 128],
                        qrhs129,
                        start=True,
                        stop=True,
                    )
                    dr_col = work.tile([128, 1], F32, tag="dr")
                    nc.vector.reciprocal(out=dr_col, in_=ps_c2q[:, 128:129])
                    nc.vector.tensor_scalar_mul(
                        gbuf[:, t, 128:256], ps_c2q[:, 0:128], dr_col
                    )
                    nc.gpsimd.tensor_mul(
                        out=gbuf[:, t, 256:384], in0=ctx_v, in1=gbuf[:, t, 128:256]
                    )
                # stream out G columns 0:384 for this group
                nc.sync.dma_start(
                    out=g_view[:, g * GS : (g + 1) * GS, 0:384],
                    in_=gbuf[:, g * GS : (g + 1) * GS, 0:384],
                )

            # ---- batch finalize: q2c ----
            u_b = batchp.tile([128, N_CTILES], F32, tag="u_b")
            nc.vector.tensor_add(out=u_b, in0=sc_buf, in1=m_buf)
            w_sel = batchp.tile([128, N_CTILES], F32, tag="w_sel")
            nc.vector.select(out=w_sel, mask=cm_tile, on_true=u_b, on_false=neg_tile)

            e_b = batchp.tile([128, N_CTILES], F32, tag="e_b")
            db_col = batchp.tile([128, 1], F32, tag="db")
            nc.scalar.activation(
                out=e_b,
                in_=w_sel,
                func=mybir.ActivationFunctionType.Exp,
                accum_out=db_col,
            )

            ps_z = pmisc.tile([1, 1], F32, tag="misc")
            nc.tensor.matmul(ps_z, db_col, ones_col, start=True, stop=True)
            zr = batchp.tile([1, 1], F32, tag="zr")
            nc.vector.reciprocal(out=zr, in_=ps_z)

            ps_q2c = pmisc.tile([128, 1], F32, tag="misc")
            for t in range(N_CTILES):
                nc.tensor.matmul(
                    ps_q2c,
                    gbuf[:, t, 0:128],
                    e_b[:, t : t + 1],
                    start=(t == 0),
                    stop=(t == N_CTILES - 1),
                )
            q2c_col = batchp.tile([128, 1], F32, tag="q2c_col")
            nc.scalar.copy(out=q2c_col, in_=ps_q2c)

            ps_q2cr = pmisc.tile([1, 128], F32, tag="misc")
            nc.tensor.transpose(ps_q2cr, q2c_col, identity)
            q2c_row = batchp.tile([1, 128], F32, tag="q2c_row")
            nc.vector.tensor_scalar_mul(q2c_row, ps_q2cr, zr)

            ps_bc = pmisc.tile([128, 128], F32, tag="misc")
            nc.tensor.matmul(ps_bc, ones_row, q2c_row, start=True, stop=True)
            bc_sb = batchp.tile([128, 128], F32, tag="bc_sb")
            nc.scalar.copy(out=bc_sb, in_=ps_bc)

            # ---- per-tile G4, stores streamed per group ----
            for t in range(N_CTILES):
                nc.gpsimd.tensor_mul(
                    out=gbuf[:, t, 384:512], in0=gbuf[:, t, 0:128], in1=bc_sb
                )
                if t % GS == GS - 1:
                    g0 = t - GS + 1
                    nc.sync.dma_start(
                        out=g_view[:, g0 : t + 1, 384:512],
                        in_=gbuf[:, g0 : t + 1, 384:512],
                    )

    return nc


_NC_CACHE = None


def _get_nc():
    global _NC_CACHE
    if _NC_CACHE is None:
        _NC_CACHE = build_nc()
    return _NC_CACHE


def kernel(context, query, W, context_mask, query_mask):
    from concourse.bass_utils import run_bass_kernel_spmd

    context = np.ascontiguousarray(np.asarray(context, dtype=np.float32))
    query = np.ascontiguousarray(np.asarray(query, dtype=np.float32))
    W = np.ascontiguousarray(np.asarray(W, dtype=np.float32))
    context_mask = np.ascontiguousarray(np.asarray(context_mask, dtype=np.int32))
    query_mask = np.ascontiguousarray(np.asarray(query_mask, dtype=np.int32))

    nc = _get_nc()
    in_maps = []
    for c in range(N_CORES):
        sl = slice(c * B_PER_CORE, (c + 1) * B_PER_CORE)
        in_maps.append(
            {
                "context": context[sl],
                "query": query[sl],
                "W": W,
                "context_mask": context_mask[sl],
                "query_mask": query_mask[sl],
            }
        )
    res = run_bass_kernel_spmd(nc, in_maps, core_ids=list(range(N_CORES)))
    out = np.concatenate([res.results[c]["G"] for c in range(N_CORES)], axis=0)
    return out.astype(np.float32)


if __name__ == "__main__":
    from concourse.timeline_sim import TimelineSim

    nc = build_nc()
    dur = TimelineSim(nc).simulate()
    print(f"TimelineSim estimated duration: {dur:.0f} ns")


# revision 17
# speedup vs baseline: 1.0771x; 1.0771x over previous
"""BiDAF attention Trainium2 kernel.

Full-input contract: kernel(**inputs) takes the unsharded tensors
(context [16,2048,128], query [16,128,128], W [384],
context_mask [16,2048] i32, query_mask [16,128] i32) and returns
G = [16, 2048, 512] f32.

Sharding: data-parallel over batch across 8 NeuronCores (2 batches/core).
Each batch's math is fully local to a core, so no collectives.
"""

import sys

sys.path.insert(0, "/opt/trn_rl_repo")

import numpy as np

import concourse.bass as bass
import concourse.tile as tile
from concourse import mybir
from concourse.masks import make_identity
from concourse.vector_clock import ScopedClock

B, C_LEN, Q_LEN, H = 16, 2048, 128, 128
NEG = -1000000000.0
N_CORES = 8
B_PER_CORE = B // N_CORES          # 2
N_CTILES = C_LEN // 128            # 16
F32 = mybir.dt.float32
I32 = mybir.dt.int32

MAX_WAITS_PER_INST = 1


def _split_excess_waits(nc, insts):
    """Hoist all but one sync wait per instruction onto preceding nops.

    The walrus build in this container rejects >1 sync wait on an
    instruction's descriptor, while Tile's sem assignment freely attaches
    several. A nop on the same engine right before the instruction stalls
    the engine identically.
    """
    out = []
    for inst in insts:
        si = getattr(inst, "sync_info", None)
        waits = list(si.on_wait) if si is not None and si.on_wait else []
        if len(waits) > MAX_WAITS_PER_INST and type(inst).__name__.startswith("Inst"):
            extra = waits[: -MAX_WAITS_PER_INST or None]
            keep = waits[-MAX_WAITS_PER_INST:]
            for i in range(0, len(extra), MAX_WAITS_PER_INST):
                out.append(
                    mybir.InstNoOp(
                        name=nc.get_next_instruction_name(),
                        sync_info=mybir.SyncInfo(
                            on_wait=extra[i : i + MAX_WAITS_PER_INST], on_update=[]
                        ),
                        bass_nofuse=True,
                        engine=inst.engine,
                    )
                )
            inst.sync_info = mybir.SyncInfo(
                on_wait=keep, on_update=list(si.on_update or [])
            )
        out.append(inst)
    return out


class SplitDrainTileContext(tile.TileContext):
    """TileContext whose tail drain splits its sem waits across SP nops.

    The walrus build in this container rejects more than one sync wait on a
    TPB_CTRL instruction; the stock tail drain carries one wait per live proc.
    """

    def _lower_ordered_insts(self, ordered):
        for bb_name in list(ordered.keys()):
            ordered[bb_name] = _split_excess_waits(self.nc, ordered[bb_name])
        return super()._lower_ordered_insts(ordered)

    def _drain_and_barrier(self, tick_clock, wait_clock):
        nc = self.nc
        drain_inst = nc.sync.drain()
        wait_clock.add_sem_waits(
            drain_inst.ins, ScopedClock({None: tick_clock.global_clock})
        )
        si = drain_inst.ins.sync_info
        waits = list(si.on_wait) if si is not None and si.on_wait else []
        if waits:
            drain_inst.ins.sync_info = mybir.SyncInfo(
                on_wait=[], on_update=list(si.on_update or [])
            )
            for i in range(0, len(waits), MAX_WAITS_PER_INST):
                nop = nc.sync.nop()
                nop.ins.sync_info = mybir.SyncInfo(
                    on_wait=waits[i : i + MAX_WAITS_PER_INST], on_update=[]
                )
        nc.all_engine_barrier()
        assert self.sems is not None
        popped = nc._tile_sem_poison_stack.pop()
        assert popped is self._sem_poison
        nc.clear_and_free_semaphores(list(self.sems.allocated().values()))
        nc.all_engine_barrier()


def build_nc() -> bass.Bass:
    nc = bass.Bass()
    ctx_d = nc.dram_tensor("context", [B_PER_CORE, C_LEN, H], F32, kind="ExternalInput")
    qry_d = nc.dram_tensor("query", [B_PER_CORE, Q_LEN, H], F32, kind="ExternalInput")
    w_d = nc.dram_tensor("W", [3 * H], F32, kind="ExternalInput")
    cm_d = nc.dram_tensor("context_mask", [B_PER_CORE, C_LEN], I32, kind="ExternalInput")
    qm_d = nc.dram_tensor("query_mask", [B_PER_CORE, Q_LEN], I32, kind="ExternalInput")
    g_d = nc.dram_tensor("G", [B_PER_CORE, C_LEN, 4 * H], F32, kind="ExternalOutput")

    F32R = mybir.dt.float32r
    NGRP = 4                      # tile groups per batch
    GS = N_CTILES // NGRP         # tiles per group (4)

    from contextlib import ExitStack

    with SplitDrainTileContext(nc) as tc, ExitStack() as es:
        consts = es.enter_context(tc.tile_pool(name="consts", bufs=1))
        batchp = es.enter_context(tc.tile_pool(name="batchp", bufs=2))
        work = es.enter_context(tc.tile_pool(name="work", bufs=4))
        gpool = es.enter_context(tc.tile_pool(name="gpool", bufs=2))
        pmm = es.enter_context(tc.tile_pool(name="pmm", bufs=2, space="PSUM"))
        ptrans = es.enter_context(tc.tile_pool(name="ptrans", bufs=2, space="PSUM"))
        pmisc = es.enter_context(tc.tile_pool(name="pmisc", bufs=2, space="PSUM"))
        ptt = es.enter_context(tc.tile_pool(name="ptt", bufs=2, space="PSUM"))

        identity = consts.tile([128, 128], F32)
        make_identity(nc, identity)
        ones_row = consts.tile([1, 128], F32)
        nc.vector.memset(ones_row, 1.0)
        ones_col = consts.tile([128, 1], F32)
        nc.vector.memset(ones_col, 1.0)
        neg_tile = consts.tile([128, N_CTILES], F32)
        nc.vector.memset(neg_tile, NEG)
        w_c = consts.tile([128, 1], F32)
        nc.sync.dma_start(out=w_c, in_=w_d[0:H].rearrange("(h o) -> h o", o=1))
        w_q = consts.tile([128, 1], F32)
        nc.sync.dma_start(out=w_q, in_=w_d[H : 2 * H].rearrange("(h o) -> h o", o=1))
        w_cq = consts.tile([128, 1], F32)
        nc.sync.dma_start(out=w_cq, in_=w_d[2 * H : 3 * H].rearrange("(h o) -> h o", o=1))

        for b in range(B_PER_CORE):
            # ---- per-batch prelims ----
            # qrhs129 = [query | 1]; the moving operand of the c2q matmul
            # (the ones column turns into the softmax denominator)
            qrhs129 = batchp.tile([128, 129], F32, tag="qrhs129")
            query = qrhs129[:, 0:128]
            nc.sync.dma_start(out=query, in_=qry_d[b])
            nc.vector.memset(qrhs129[:, 128:129], 1.0)

            ps_qt = ptrans.tile([128, 128], F32, tag="tr")
            nc.tensor.transpose(ps_qt, query, identity)
            queryT = batchp.tile([128, 128], F32, tag="queryT")
            nc.scalar.copy(out=queryT, in_=ps_qt)

            # rhs129 = [queryT * w_cq | w_c]  (moving operand of the S matmul;
            # its first 128 cols are also the T^T stationary operand)
            rhs129 = batchp.tile([128, 129], F32, tag="rhs129")
            qTw = rhs129[:, 0:128]
            nc.vector.tensor_scalar_mul(qTw, queryT, w_cq)
            nc.vector.tensor_copy(out=rhs129[:, 128:129], in_=w_c)

            # sqm as a column [q, 1]:  s_q + (qm - 1) * 1e9
            ps_sqc = pmisc.tile([128, 1], F32, tag="misc")
            nc.tensor.matmul(ps_sqc, queryT, w_q, start=True, stop=True)
            qm_col = batchp.tile([128, 1], I32, tag="qm_col")
            nc.sync.dma_start(out=qm_col, in_=qm_d[b].rearrange("(q o) -> q o", o=1))
            qoff = batchp.tile([128, 1], F32, tag="qoff")
            nc.vector.tensor_copy(out=qoff, in_=qm_col)
            nc.vector.tensor_scalar(
                out=qoff,
                in0=qoff,
                scalar1=1.0,
                scalar2=-NEG,
                op0=mybir.AluOpType.subtract,
                op1=mybir.AluOpType.mult,
            )
            sqm_col = batchp.tile([128, 1], F32, tag="sqm_col")
            nc.vector.tensor_add(out=sqm_col, in0=ps_sqc, in1=qoff)

            # sqm as a row, padded to 129 cols; accumulated into the S matmul
            # via a k=1 PSUM-accumulate so T already carries s_q + mask
            ps_sqr = ptrans.tile([1, 128], F32, tag="tr")
            nc.tensor.transpose(ps_sqr, sqm_col, identity)
            srow129 = batchp.tile([1, 129], F32, tag="srow129")
            nc.vector.memset(srow129[:, 128:129], 0.0)
            nc.vector.tensor_copy(out=srow129[:, 0:128], in_=ps_sqr)

            cm_tile = batchp.tile([128, N_CTILES], I32, tag="cm_tile")
            nc.sync.dma_start(
                out=cm_tile, in_=cm_d[b].rearrange("(t p) -> p t", p=128)
            )

            m_buf = batchp.tile([128, N_CTILES], F32, tag="m_buf")
            sc_buf = batchp.tile([128, N_CTILES], F32, tag="sc_buf")
            ctxT_buf = batchp.tile([128, C_LEN], F32, tag="ctxT_buf")
            eT_buf = batchp.tile([128, C_LEN], F32, tag="eT_buf")

            # one batch-wide G buffer [p, t, f]; context lands in f=0:128 in
            # two half-batch DMAs (c = t*128 + p) so compute starts earlier
            gbuf = gpool.tile([128, N_CTILES, 4 * H], F32, tag="g")
            ctx_view = ctx_d[b].rearrange("(t p) h -> p t h", p=128)
            hh = N_CTILES // 2
            nc.sync.dma_start(out=gbuf[:, 0:hh, 0:128], in_=ctx_view[:, 0:hh, :])
            nc.sync.dma_start(out=gbuf[:, hh:, 0:128], in_=ctx_view[:, hh:, :])
            g_view = g_d[b].rearrange("(t p) f -> p t f", p=128)

            for g in range(NGRP):
                # ---- phase A per tile: transpose ctx, S matmul, row-max ----
                for t in range(g * GS, (g + 1) * GS):
                    ctx_v = gbuf[:, t, 0:128]
                    ps_ct = ptrans.tile([128, 128], F32, tag="tr")
                    nc.tensor.transpose(ps_ct, ctx_v, identity)
                    ctxT = ctxT_buf[:, t * 128 : (t + 1) * 128]
                    nc.any.tensor_copy(out=ctxT, in_=ps_ct)

                    ps_T = pmm.tile([128, 129], F32, tag="mm")
                    nc.tensor.matmul(ps_T, ctxT, rhs129, start=True, stop=False)
                    nc.tensor.matmul(ps_T, ones_row, srow129, start=False, stop=True)
                    nc.vector.tensor_copy(
                        out=sc_buf[:, t : t + 1], in_=ps_T[:, 128:129]
                    )
                    nc.vector.reduce_max(
                        out=m_buf[:, t : t + 1],
                        in_=ps_T[:, 0:128],
                        axis=mybir.AxisListType.X,
                    )

                # ---- T^T chunk (fp32r) + exp -> eT ----
                c0 = g * GS * 128
                ps_tt = ptt.tile([128, GS * 128], F32, tag="tt")
                nc.tensor.matmul(
                    ps_tt,
                    qTw.bitcast(F32R),
                    ctxT_buf[:, c0 : c0 + GS * 128].bitcast(F32R),
                    start=True,
                    stop=True,
                )
                nc.scalar.activation(
                    out=eT_buf[:, c0 : c0 + GS * 128],
                    in_=ps_tt,
                    func=mybir.ActivationFunctionType.Exp,
                    bias=sqm_col,
                )

                # ---- phase B per tile: c2q, normalize, ctx*c2q ----
                for t in range(g * GS, (g + 1) * GS):
                    ctx_v = gbuf[:, t, 0:128]
                    ps_c2q = pmm.tile([128, 129], F32, tag="mm")
                    nc.tensor.matmul(
                        ps_c2q,
                        eT_buf[:, t * 128 : (t + 1) * 128],
                        qrhs129,
                        start=True,
                        stop=True,
                    )
                    dr_col = work.tile([128, 1], F32, tag="dr")
                    nc.vector.reciprocal(out=dr_col, in_=ps_c2q[:, 128:129])
                    nc.vector.tensor_scalar_mul(
                        gbuf[:, t, 128:256], ps_c2q[:, 0:128], dr_col
                    )
                    nc.gpsimd.tensor_mul(
                        out=gbuf[:, t, 256:384], in0=ctx_v, in1=gbuf[:, t, 128:256]
                    )
                # stream out G columns 0:384 for this group
                nc.sync.dma_start(
                    out=g_view[:, g * GS : (g + 1) * GS, 0:384],
                    in_=gbuf[:, g * GS : (g + 1) * GS, 0:384],
                )

            # ---- batch finalize: q2c ----
            u_b = batchp.tile([128, N_CTILES], F32, tag="u_b")
            nc.vector.tensor_add(out=u_b, in0=sc_buf, in1=m_buf)
            w_sel = batchp.tile([128, N_CTILES], F32, tag="w_sel")
            nc.vector.select(out=w_sel, mask=cm_tile, on_true=u_b, on_false=neg_tile)

            e_b = batchp.tile([128, N_CTILES], F32, tag="e_b")
            db_col = batchp.tile([128, 1], F32, tag="db")
            nc.scalar.activation(
                out=e_b,
                in_=w_sel,
                func=mybir.ActivationFunctionType.Exp,
                accum_out=db_col,
            )

            ps_z = pmisc.tile([1, 1], F32, tag="misc")
            nc.tensor.matmul(ps_z, db_col, ones_col, start=True, stop=True)
            zr = batchp.tile([1, 1], F32, tag="zr")
            nc.vector.reciprocal(out=zr, in_=ps_z)

            ps_q2c = pmisc.tile([128, 1], F32, tag="misc")
            for t in range(N_CTILES):
                nc.tensor.matmul(
                    ps_q2c,
                    gbuf[:, t, 0:128],
                    e_b[:, t : t + 1],
                    start=(t == 0),
                    stop=(t == N_CTILES - 1),
                )
            q2c_col = batchp.tile([128, 1], F32, tag="q2c_col")
            nc.scalar.copy(out=q2c_col, in_=ps_q2c)

            ps_q2cr = pmisc.tile([1, 128], F32, tag="misc")
            nc.tensor.transpose(ps_q2cr, q2c_col, identity)
            q2c_row = batchp.tile([1, 128], F32, tag="q2c_row")
            nc.vector.tensor_scalar_mul(q2c_row, ps_q2cr, zr)

            ps_bc = pmisc.tile([128, 128], F32, tag="misc")
            nc.tensor.matmul(ps_bc, ones_row, q2c_row, start=True, stop=True)
            bc_sb = batchp.tile([128, 128], F32, tag="bc_sb")
            nc.scalar.copy(out=bc_sb, in_=ps_bc)

            # ---- per-tile G4, stores streamed per group ----
            for t in range(N_CTILES):
                nc.gpsimd.tensor_mul(
                    out=gbuf[:, t, 384:512], in0=gbuf[:, t, 0:128], in1=bc_sb
                )
                if t % GS == GS - 1:
                    g0 = t - GS + 1
                    nc.sync.dma_start(
                        out=g_view[:, g0 : t + 1, 384:512],
                        in_=gbuf[:, g0 : t + 1, 384:512],
                    )

    return nc


_NC_CACHE = None


def _get_nc():
    global _NC_CACHE
    if _NC_CACHE is None:
        _NC_CACHE = build_nc()
    return _NC_CACHE


def kernel(context, query, W, context_mask, query_mask):
    from concourse.bass_utils import run_bass_kernel_spmd

    context = np.ascontiguousarray(np.asarray(context, dtype=np.float32))
    query = np.ascontiguousarray(np.asarray(query, dtype=np.float32))
    W = np.ascontiguousarray(np.asarray(W, dtype=np.float32))
    context_mask = np.ascontiguousarray(np.asarray(context_mask, dtype=np.int32))
    query_mask = np.ascontiguousarray(np.asarray(query_mask, dtype=np.int32))

    nc = _get_nc()
    in_maps = []
    for c in range(N_CORES):
        sl = slice(c * B_PER_CORE, (c + 1) * B_PER_CORE)
        in_maps.append(
            {
                "context": context[sl],
                "query": query[sl],
                "W": W,
                "context_mask": context_mask[sl],
                "query_mask": query_mask[sl],
            }
        )
    res = run_bass_kernel_spmd(nc, in_maps, core_ids=list(range(N_CORES)))
    out = np.concatenate([res.results[c]["G"] for c in range(N_CORES)], axis=0)
    return out.astype(np.float32)


if __name__ == "__main__":
    from concourse.timeline_sim import TimelineSim

    nc = build_nc()
    dur = TimelineSim(nc).simulate()
    print(f"TimelineSim estimated duration: {dur:.0f} ns")


# revision 19
# speedup vs baseline: 1.0810x; 1.0037x over previous
"""BiDAF attention Trainium2 kernel.

Full-input contract: kernel(**inputs) takes the unsharded tensors
(context [16,2048,128], query [16,128,128], W [384],
context_mask [16,2048] i32, query_mask [16,128] i32) and returns
G = [16, 2048, 512] f32.

Sharding: data-parallel over batch across 8 NeuronCores (2 batches/core).
Each batch's math is fully local to a core, so no collectives.
"""

import sys

sys.path.insert(0, "/opt/trn_rl_repo")

import numpy as np

import concourse.bass as bass
import concourse.tile as tile
from concourse import mybir
from concourse.masks import make_identity
from concourse.vector_clock import ScopedClock

B, C_LEN, Q_LEN, H = 16, 2048, 128, 128
NEG = -1000000000.0
N_CORES = 8
B_PER_CORE = B // N_CORES          # 2
N_CTILES = C_LEN // 128            # 16
F32 = mybir.dt.float32
I32 = mybir.dt.int32

MAX_WAITS_PER_INST = 1


def _split_excess_waits(nc, insts):
    """Hoist all but one sync wait per instruction onto preceding nops.

    The walrus build in this container rejects >1 sync wait on an
    instruction's descriptor, while Tile's sem assignment freely attaches
    several. A nop on the same engine right before the instruction stalls
    the engine identically.
    """
    out = []
    for inst in insts:
        si = getattr(inst, "sync_info", None)
        waits = list(si.on_wait) if si is not None and si.on_wait else []
        if len(waits) > MAX_WAITS_PER_INST and type(inst).__name__.startswith("Inst"):
            extra = waits[: -MAX_WAITS_PER_INST or None]
            keep = waits[-MAX_WAITS_PER_INST:]
            for i in range(0, len(extra), MAX_WAITS_PER_INST):
                out.append(
                    mybir.InstNoOp(
                        name=nc.get_next_instruction_name(),
                        sync_info=mybir.SyncInfo(
                            on_wait=extra[i : i + MAX_WAITS_PER_INST], on_update=[]
                        ),
                        bass_nofuse=True,
                        engine=inst.engine,
                    )
                )
            inst.sync_info = mybir.SyncInfo(
                on_wait=keep, on_update=list(si.on_update or [])
            )
        out.append(inst)
    return out


class SplitDrainTileContext(tile.TileContext):
    """TileContext whose tail drain splits its sem waits across SP nops.

    The walrus build in this container rejects more than one sync wait on a
    TPB_CTRL instruction; the stock tail drain carries one wait per live proc.
    """

    def _lower_ordered_insts(self, ordered):
        for bb_name in list(ordered.keys()):
            ordered[bb_name] = _split_excess_waits(self.nc, ordered[bb_name])
        return super()._lower_ordered_insts(ordered)

    def _drain_and_barrier(self, tick_clock, wait_clock):
        nc = self.nc
        drain_inst = nc.sync.drain()
        wait_clock.add_sem_waits(
            drain_inst.ins, ScopedClock({None: tick_clock.global_clock})
        )
        si = drain_inst.ins.sync_info
        waits = list(si.on_wait) if si is not None and si.on_wait else []
        if waits:
            drain_inst.ins.sync_info = mybir.SyncInfo(
                on_wait=[], on_update=list(si.on_update or [])
            )
            for i in range(0, len(waits), MAX_WAITS_PER_INST):
                nop = nc.sync.nop()
                nop.ins.sync_info = mybir.SyncInfo(
                    on_wait=waits[i : i + MAX_WAITS_PER_INST], on_update=[]
                )
        nc.all_engine_barrier()
        assert self.sems is not None
        popped = nc._tile_sem_poison_stack.pop()
        assert popped is self._sem_poison
        nc.clear_and_free_semaphores(list(self.sems.allocated().values()))
        nc.all_engine_barrier()


def build_nc() -> bass.Bass:
    nc = bass.Bass()
    ctx_d = nc.dram_tensor("context", [B_PER_CORE, C_LEN, H], F32, kind="ExternalInput")
    qry_d = nc.dram_tensor("query", [B_PER_CORE, Q_LEN, H], F32, kind="ExternalInput")
    w_d = nc.dram_tensor("W", [3 * H], F32, kind="ExternalInput")
    cm_d = nc.dram_tensor("context_mask", [B_PER_CORE, C_LEN], I32, kind="ExternalInput")
    qm_d = nc.dram_tensor("query_mask", [B_PER_CORE, Q_LEN], I32, kind="ExternalInput")
    g_d = nc.dram_tensor("G", [B_PER_CORE, C_LEN, 4 * H], F32, kind="ExternalOutput")

    F32R = mybir.dt.float32r
    NGRP = 4                      # tile groups per batch
    GS = N_CTILES // NGRP         # tiles per group (4)

    from contextlib import ExitStack

    with SplitDrainTileContext(nc) as tc, ExitStack() as es:
        consts = es.enter_context(tc.tile_pool(name="consts", bufs=1))
        batchp = es.enter_context(tc.tile_pool(name="batchp", bufs=2))
        work = es.enter_context(tc.tile_pool(name="work", bufs=4))
        gpool = es.enter_context(tc.tile_pool(name="gpool", bufs=2))
        pmm = es.enter_context(tc.tile_pool(name="pmm", bufs=2, space="PSUM"))
        ptrans = es.enter_context(tc.tile_pool(name="ptrans", bufs=2, space="PSUM"))
        pmisc = es.enter_context(tc.tile_pool(name="pmisc", bufs=2, space="PSUM"))
        ptt = es.enter_context(tc.tile_pool(name="ptt", bufs=2, space="PSUM"))

        identity = consts.tile([128, 128], F32)
        make_identity(nc, identity)
        ones_row = consts.tile([1, 128], F32)
        nc.vector.memset(ones_row, 1.0)
        ones_col = consts.tile([128, 1], F32)
        nc.vector.memset(ones_col, 1.0)
        neg_tile = consts.tile([128, N_CTILES], F32)
        nc.vector.memset(neg_tile, NEG)
        w_c = consts.tile([128, 1], F32)
        nc.sync.dma_start(out=w_c, in_=w_d[0:H].rearrange("(h o) -> h o", o=1))
        w_q = consts.tile([128, 1], F32)
        nc.sync.dma_start(out=w_q, in_=w_d[H : 2 * H].rearrange("(h o) -> h o", o=1))
        w_cq = consts.tile([128, 1], F32)
        nc.sync.dma_start(out=w_cq, in_=w_d[2 * H : 3 * H].rearrange("(h o) -> h o", o=1))

        for b in range(B_PER_CORE):
            # ---- per-batch prelims ----
            # qrhs129 = [query | 1]; the moving operand of the c2q matmul
            # (the ones column turns into the softmax denominator)
            qrhs129 = batchp.tile([128, 129], F32, tag="qrhs129")
            query = qrhs129[:, 0:128]
            nc.sync.dma_start(out=query, in_=qry_d[b])
            nc.vector.memset(qrhs129[:, 128:129], 1.0)

            ps_qt = ptrans.tile([128, 128], F32, tag="tr")
            nc.tensor.transpose(ps_qt, query, identity)
            queryT = batchp.tile([128, 128], F32, tag="queryT")
            nc.scalar.copy(out=queryT, in_=ps_qt)

            # rhs129 = [queryT * w_cq | w_c]  (moving operand of the S matmul;
            # its first 128 cols are also the T^T stationary operand)
            rhs129 = batchp.tile([128, 129], F32, tag="rhs129")
            qTw = rhs129[:, 0:128]
            nc.vector.tensor_scalar_mul(qTw, queryT, w_cq)
            nc.vector.tensor_copy(out=rhs129[:, 128:129], in_=w_c)

            # sqm as a column [q, 1]:  s_q + (qm - 1) * 1e9
            ps_sqc = pmisc.tile([128, 1], F32, tag="misc")
            nc.tensor.matmul(ps_sqc, queryT, w_q, start=True, stop=True)
            qm_col = batchp.tile([128, 1], I32, tag="qm_col")
            nc.sync.dma_start(out=qm_col, in_=qm_d[b].rearrange("(q o) -> q o", o=1))
            qoff = batchp.tile([128, 1], F32, tag="qoff")
            nc.vector.tensor_copy(out=qoff, in_=qm_col)
            nc.vector.tensor_scalar(
                out=qoff,
                in0=qoff,
                scalar1=1.0,
                scalar2=-NEG,
                op0=mybir.AluOpType.subtract,
                op1=mybir.AluOpType.mult,
            )
            sqm_col = batchp.tile([128, 1], F32, tag="sqm_col")
            nc.vector.tensor_add(out=sqm_col, in0=ps_sqc, in1=qoff)

            # sqm as a row, padded to 129 cols; accumulated into the S matmul
            # via a k=1 PSUM-accumulate so T already carries s_q + mask
            ps_sqr = ptrans.tile([1, 128], F32, tag="tr")
            nc.tensor.transpose(ps_sqr, sqm_col, identity)
            srow129 = batchp.tile([1, 129], F32, tag="srow129")
            nc.vector.memset(srow129[:, 128:129], 0.0)
            nc.vector.tensor_copy(out=srow129[:, 0:128], in_=ps_sqr)

            cm_tile = batchp.tile([128, N_CTILES], I32, tag="cm_tile")
            nc.sync.dma_start(
                out=cm_tile, in_=cm_d[b].rearrange("(t p) -> p t", p=128)
            )

            m_buf = batchp.tile([128, N_CTILES], F32, tag="m_buf")
            sc_buf = batchp.tile([128, N_CTILES], F32, tag="sc_buf")
            ctxT_buf = batchp.tile([128, C_LEN], F32, tag="ctxT_buf")
            eT_buf = batchp.tile([128, C_LEN], F32, tag="eT_buf")

            # one batch-wide G buffer [p, t, f]; context lands in f=0:128 in
            # two half-batch DMAs (c = t*128 + p) so compute starts earlier
            gbuf = gpool.tile([128, N_CTILES, 4 * H], F32, tag="g")
            ctx_view = ctx_d[b].rearrange("(t p) h -> p t h", p=128)
            hh = N_CTILES // 2
            nc.sync.dma_start(out=gbuf[:, 0:hh, 0:128], in_=ctx_view[:, 0:hh, :])
            nc.sync.dma_start(out=gbuf[:, hh:, 0:128], in_=ctx_view[:, hh:, :])
            g_view = g_d[b].rearrange("(t p) f -> p t f", p=128)

            u_b = batchp.tile([128, N_CTILES], F32, tag="u_b")
            e_b = batchp.tile([128, N_CTILES], F32, tag="e_b")
            ps_q2c = pmisc.tile([128, 1], F32, tag="misc")

            for g in range(NGRP):
                # ---- phase A per tile: transpose ctx, S matmul, row-max ----
                for t in range(g * GS, (g + 1) * GS):
                    ctx_v = gbuf[:, t, 0:128]
                    ps_ct = ptrans.tile([128, 128], F32, tag="tr")
                    nc.tensor.transpose(ps_ct, ctx_v, identity)
                    ctxT = ctxT_buf[:, t * 128 : (t + 1) * 128]
                    nc.any.tensor_copy(out=ctxT, in_=ps_ct)

                    ps_T = pmm.tile([128, 129], F32, tag="mm")
                    nc.tensor.matmul(ps_T, ctxT, rhs129, start=True, stop=False)
                    nc.tensor.matmul(ps_T, ones_row, srow129, start=False, stop=True)
                    nc.vector.tensor_copy(
                        out=sc_buf[:, t : t + 1], in_=ps_T[:, 128:129]
                    )
                    nc.vector.reduce_max(
                        out=m_buf[:, t : t + 1],
                        in_=ps_T[:, 0:128],
                        axis=mybir.AxisListType.X,
                    )

                # ---- T^T chunk (fp32r) + exp -> eT ----
                c0 = g * GS * 128
                ps_tt = ptt.tile([128, GS * 128], F32, tag="tt")
                nc.tensor.matmul(
                    ps_tt,
                    qTw.bitcast(F32R),
                    ctxT_buf[:, c0 : c0 + GS * 128].bitcast(F32R),
                    start=True,
                    stop=True,
                )
                nc.scalar.activation(
                    out=eT_buf[:, c0 : c0 + GS * 128],
                    in_=ps_tt,
                    func=mybir.ActivationFunctionType.Exp,
                    bias=sqm_col,
                )

                # ---- phase B per tile: c2q, normalize, ctx*c2q ----
                for t in range(g * GS, (g + 1) * GS):
                    ctx_v = gbuf[:, t, 0:128]
                    ps_c2q = pmm.tile([128, 129], F32, tag="mm")
                    nc.tensor.matmul(
                        ps_c2q,
                        eT_buf[:, t * 128 : (t + 1) * 128],
                        qrhs129,
                        start=True,
                        stop=True,
                    )
                    dr_col = work.tile([128, 1], F32, tag="dr")
                    nc.vector.reciprocal(out=dr_col, in_=ps_c2q[:, 128:129])
                    nc.vector.tensor_scalar_mul(
                        gbuf[:, t, 128:256], ps_c2q[:, 0:128], dr_col
                    )
                    nc.gpsimd.tensor_mul(
                        out=gbuf[:, t, 256:384], in0=ctx_v, in1=gbuf[:, t, 128:256]
                    )
                # stream out G columns 0:384 for this group
                nc.sync.dma_start(
                    out=g_view[:, g * GS : (g + 1) * GS, 0:384],
                    in_=gbuf[:, g * GS : (g + 1) * GS, 0:384],
                )

                # ---- incremental q2c: this group's b-softmax logits + mms ----
                gs_sl = slice(g * GS, (g + 1) * GS)
                nc.vector.tensor_add(
                    out=u_b[:, gs_sl], in0=sc_buf[:, gs_sl], in1=m_buf[:, gs_sl]
                )
                nc.vector.select(
                    out=e_b[:, gs_sl],
                    mask=cm_tile[:, gs_sl],
                    on_true=u_b[:, gs_sl],
                    on_false=neg_tile[:, gs_sl],
                )
                nc.scalar.activation(
                    out=e_b[:, gs_sl],
                    in_=e_b[:, gs_sl],
                    func=mybir.ActivationFunctionType.Exp,
                )
                for t in range(g * GS, (g + 1) * GS):
                    nc.tensor.matmul(
                        ps_q2c,
                        gbuf[:, t, 0:128],
                        e_b[:, t : t + 1],
                        start=(t == 0),
                        stop=(t == N_CTILES - 1),
                    )

            # ---- batch finalize: Z, q2c row, broadcast ----
            ps_z = pmisc.tile([1, N_CTILES], F32, tag="misc")
            nc.tensor.matmul(ps_z, ones_col, e_b, start=True, stop=True)
            z_tot = batchp.tile([1, 1], F32, tag="z_tot")
            nc.vector.reduce_sum(out=z_tot, in_=ps_z, axis=mybir.AxisListType.X)
            zr = batchp.tile([1, 1], F32, tag="zr")
            nc.vector.reciprocal(out=zr, in_=z_tot)

            q2c_col = batchp.tile([128, 1], F32, tag="q2c_col")
            nc.scalar.copy(out=q2c_col, in_=ps_q2c)

            ps_q2cr = pmisc.tile([1, 128], F32, tag="misc")
            nc.tensor.transpose(ps_q2cr, q2c_col, identity)
            q2c_row = batchp.tile([1, 128], F32, tag="q2c_row")
            nc.vector.tensor_scalar_mul(q2c_row, ps_q2cr, zr)

            ps_bc = pmisc.tile([128, 128], F32, tag="misc")
            nc.tensor.matmul(ps_bc, ones_row, q2c_row, start=True, stop=True)
            bc_sb = batchp.tile([128, 128], F32, tag="bc_sb")
            nc.scalar.copy(out=bc_sb, in_=ps_bc)

            # ---- per-tile G4, stores streamed per group ----
            for t in range(N_CTILES):
                nc.gpsimd.tensor_mul(
                    out=gbuf[:, t, 384:512], in0=gbuf[:, t, 0:128], in1=bc_sb
                )
                if t % GS == GS - 1:
                    g0 = t - GS + 1
                    nc.sync.dma_start(
                        out=g_view[:, g0 : t + 1, 384:512],
                        in_=gbuf[:, g0 : t + 1, 384:512],
                    )

    return nc


_NC_CACHE = None


def _get_nc():
    global _NC_CACHE
    if _NC_CACHE is None:
        _NC_CACHE = build_nc()
    return _NC_CACHE


def kernel(context, query, W, context_mask, query_mask):
    from concourse.bass_utils import run_bass_kernel_spmd

    context = np.ascontiguousarray(np.asarray(context, dtype=np.float32))
    query = np.ascontiguousarray(np.asarray(query, dtype=np.float32))
    W = np.ascontiguousarray(np.asarray(W, dtype=np.float32))
    context_mask = np.ascontiguousarray(np.asarray(context_mask, dtype=np.int32))
    query_mask = np.ascontiguousarray(np.asarray(query_mask, dtype=np.int32))

    nc = _get_nc()
    in_maps = []
    for c in range(N_CORES):
        sl = slice(c * B_PER_CORE, (c + 1) * B_PER_CORE)
        in_maps.append(
            {
                "context": context[sl],
                "query": query[sl],
                "W": W,
                "context_mask": context_mask[sl],
                "query_mask": query_mask[sl],
            }
        )
    res = run_bass_kernel_spmd(nc, in_maps, core_ids=list(range(N_CORES)))
    out = np.concatenate([res.results[c]["G"] for c in range(N_CORES)], axis=0)
    return out.astype(np.float32)


if __name__ == "__main__":
    from concourse.timeline_sim import TimelineSim

    nc = build_nc()
    dur = TimelineSim(nc).simulate()
    print(f"TimelineSim estimated duration: {dur:.0f} ns")
